# revision 5
# baseline (speedup 1.0000x reference)
"""Trainium2 Bass kernel: single-head causal attention, data-parallel over batch.

Reference computation (per batch b):
    q = x @ Wq + bq; k = x @ Wk + bk; v = x @ Wv + bv       # [S, F]
    s = (q @ k^T) / sqrt(D)   (causal masked)               # [S, S]
    p = softmax(s, axis=-1)
    out = p @ v                                             # [S, F]

Shapes: B=32, S=1024, D=1024, F=64.  8 cores, 4 batches/core.

Device-side layout strategy (per core, per batch):
  - host provides x pre-transposed: XT[d, s] (bf16), chunked [8, 128, 1024]
  - pass1: psum_qk[128, S] = [Wq | Wk]^T-chunks (stationary, M=128) vs XT (moving)
           -> partitions 0-63 = Q^T, 64-127 = K^T  (full-rate M=128)
  - pass2: psum_v[64, S]  = Wv-chunks vs XT  -> V^T
  - V^T -> V via 8 PE transposes; V augmented with a ones column (M=65)
  - scores computed TRANSPOSED: ST[k, q] = K^T-chunk (stationary) vs Q^T (moving)
    so softmax probs P^T = exp(ST) feed the P@V matmul directly as moving operand
  - causal: only q >= 128*t computed per k-chunk t; diagonal 128x128 tile
    masked by multiplying exp() output with an upper-triangular 0/1 mask (GPSIMD)
  - P@V: out_aug[65, q] accumulates over k-chunks; row 64 = sum(exp) (softmax denom)
  - host divides by the denominator row and transposes back.
"""

import numpy as np
import ml_dtypes

BF16 = ml_dtypes.bfloat16

B, S, D, F = 32, 1024, 1024, 64
NCORES = 8
BPC = B // NCORES          # batches per core
NCH = D // 128             # contraction chunks (8)

_CACHE = {}


def _build_program():
    from contextlib import ExitStack

    import concourse.bacc as bacc
    import concourse.tile as tile
    from concourse import mybir

    f32 = mybir.dt.float32
    bf16 = mybir.dt.bfloat16
    Exp = mybir.ActivationFunctionType.Exp

    nc = bacc.Bacc(None, target_bir_lowering=False, debug=False)

    xt_d = nc.declare_dram_parameter("xt", [BPC, NCH, 128, S], bf16, isOutput=False)
    wqk_d = nc.declare_dram_parameter("wqk", [NCH, 128, 128], bf16, isOutput=False)
    wv_d = nc.declare_dram_parameter("wv", [NCH, 128, F], bf16, isOutput=False)
    bqk_d = nc.declare_dram_parameter("bqk", [128, 1], f32, isOutput=False)
    bv_d = nc.declare_dram_parameter("bv", [F, 1], f32, isOutput=False)
    tri_d = nc.declare_dram_parameter("tri", [128, 128], bf16, isOutput=False)
    ident_d = nc.declare_dram_parameter("ident", [F, F], bf16, isOutput=False)
    out_d = nc.declare_dram_parameter("out", [BPC, F + 1, S], f32, isOutput=True)

    with tile.TileContext(nc) as tc, ExitStack() as ctx:
        singles = ctx.enter_context(tc.tile_pool(name="singles", bufs=1))
        xt_pool = ctx.enter_context(tc.tile_pool(name="xt", bufs=2))
        qkv_pool = ctx.enter_context(tc.tile_pool(name="qkv", bufs=2))
        pt_pool = ctx.enter_context(tc.tile_pool(name="pt", bufs=3))
        osb_pool = ctx.enter_context(tc.tile_pool(name="osb", bufs=2))
        pp_pool = ctx.enter_context(tc.tile_pool(name="pp", bufs=2, space="PSUM"))
        po_pool = ctx.enter_context(tc.tile_pool(name="po", bufs=2, space="PSUM"))

        # --- load constants / weights once ---
        wqk_sb = singles.tile([128, NCH, 128], bf16, tag="wqk")
        nc.sync.dma_start(out=wqk_sb, in_=wqk_d[:, :, :].rearrange("c p m -> p c m"))
        wv_sb = singles.tile([128, NCH, F], bf16, tag="wv")
        nc.sync.dma_start(out=wv_sb, in_=wv_d[:, :, :].rearrange("c p m -> p c m"))
        bqk_sb = singles.tile([128, 1], f32, tag="bqk")
        nc.sync.dma_start(out=bqk_sb, in_=bqk_d[:, :])
        bv_sb = singles.tile([F, 1], f32, tag="bv")
        nc.sync.dma_start(out=bv_sb, in_=bv_d[:, :])
        tri_sb = singles.tile([128, 128], bf16, tag="tri")
        nc.sync.dma_start(out=tri_sb, in_=tri_d[:, :])
        ident_sb = singles.tile([F, F], bf16, tag="ident")
        nc.sync.dma_start(out=ident_sb, in_=ident_d[:, :])

        for b in range(BPC):
            # --- load XT for this batch (two halves so matmuls start earlier) ---
            xt = xt_pool.tile([128, NCH, S], bf16, tag="xt")
            nc.sync.dma_start(
                out=xt[:, :, 0:512],
                in_=xt_d[b, :, :, 0:512].rearrange("c p s -> p c s"),
            )
            nc.sync.dma_start(
                out=xt[:, :, 512:1024],
                in_=xt_d[b, :, :, 512:1024].rearrange("c p s -> p c s"),
            )

            # --- pass1: Q^T (parts 0-63) and K^T (parts 64-127) ---
            psum_qk = pp_pool.tile([128, S], f32, tag="pp")
            for sj in range(2):
                for c in range(NCH):
                    nc.tensor.matmul(
                        out=psum_qk[:, 512 * sj : 512 * sj + 512],
                        lhsT=wqk_sb[:, c, :],
                        rhs=xt[:, c, 512 * sj : 512 * sj + 512],
                        start=(c == 0),
                        stop=(c == NCH - 1),
                    )
            qt = qkv_pool.tile([64, S], bf16, tag="qt")
            nc.vector.tensor_scalar_add(out=qt, in0=psum_qk[0:64, :], scalar1=bqk_sb[0:64, :])
            kt = qkv_pool.tile([64, S], bf16, tag="kt")
            nc.vector.tensor_scalar_add(out=kt, in0=psum_qk[64:128, :], scalar1=bqk_sb[64:128, :])

            # --- pass2: V^T ---
            psum_v = pp_pool.tile([64, S], f32, tag="pp")
            for sj in range(2):
                for c in range(NCH):
                    nc.tensor.matmul(
                        out=psum_v[:, 512 * sj : 512 * sj + 512],
                        lhsT=wv_sb[:, c, :],
                        rhs=xt[:, c, 512 * sj : 512 * sj + 512],
                        start=(c == 0),
                        stop=(c == NCH - 1),
                    )
            vt = qkv_pool.tile([64, S], bf16, tag="vt")
            nc.vector.tensor_scalar_add(out=vt, in0=psum_v[:, :], scalar1=bv_sb)

            # --- V^T -> V (PE transposes), augmented with ones column ---
            psum_vt = pp_pool.tile([128, NCH, F], bf16, tag="pp")
            for t in range(NCH):
                nc.tensor.transpose(
                    out=psum_vt[:, t, :],
                    in_=vt[:, 128 * t : 128 * t + 128],
                    identity=ident_sb,
                )
            v_aug = qkv_pool.tile([128, NCH, F + 1], bf16, tag="vaug")
            nc.gpsimd.memset(v_aug[:, :, F : F + 1], 1.0)
            nc.vector.tensor_copy(out=v_aug[:, :, 0:F], in_=psum_vt[:, :, :])

            # --- attention: k-chunk t covers queries q >= 128*t ---
            psum_o = po_pool.tile([F + 1, S], f32, tag="po")
            for t in range(NCH):
                q0 = 128 * t
                L = S - q0
                segs = []
                for j in range(2):
                    g0 = max(512 * j, q0)
                    g1 = 512 * (j + 1)
                    if g0 < g1:
                        segs.append((j, g0, g1))

                psum_s = pp_pool.tile([128, S], f32, tag="pp")
                for (_, g0, g1) in segs:
                    nc.tensor.matmul(
                        out=psum_s[:, g0:g1],
                        lhsT=kt[:, q0 : q0 + 128],
                        rhs=qt[:, g0:g1],
                        start=True,
                        stop=True,
                    )
                pt = pt_pool.tile([128, S], bf16, tag="pt")
                nc.scalar.activation(out=pt[:, 0:L], in_=psum_s[:, q0:S], func=Exp)
                # causal mask on the diagonal 128x128 tile (upper-tri 0/1)
                nc.gpsimd.tensor_mul(
                    out=pt[:, 0:128], in0=pt[:, 0:128], in1=tri_sb
                )
                for (j, g0, g1) in segs:
                    nc.tensor.matmul(
                        out=psum_o[:, g0:g1],
                        lhsT=v_aug[:, t, :],
                        rhs=pt[:, g0 - q0 : g1 - q0],
                        start=(t == 0),
                        stop=(t == (3 if j == 0 else 7)),
                    )

            out_sb = osb_pool.tile([F + 1, S], f32, tag="osb")
            nc.vector.tensor_copy(out=out_sb, in_=psum_o)
            nc.sync.dma_start(out=out_d[b], in_=out_sb)

    if not nc.is_finalized():
        nc.finalize()
    return nc


def _prep_shared(Wq, bq, Wk, bk, Wv, bv):
    scale = 1.0 / np.sqrt(np.float32(D))
    wqks = np.concatenate([Wq.astype(np.float32) * scale, Wk.astype(np.float32)], axis=1)  # [D, 128]
    wqk = np.ascontiguousarray(
        wqks.reshape(NCH, 128, 128)
    ).astype(BF16)  # [c, p, m]
    wv = np.ascontiguousarray(
        Wv.astype(np.float32).reshape(NCH, 128, F)
    ).astype(BF16)
    bqk = np.concatenate(
        [bq.astype(np.float32) * scale, bk.astype(np.float32)]
    ).reshape(128, 1)
    bvv = bv.astype(np.float32).reshape(F, 1)
    tri = np.triu(np.ones((128, 128), np.float32)).astype(BF16)
    ident = np.eye(F, dtype=np.float32).astype(BF16)
    return wqk, wv, bqk, bvv, tri, ident


def _ensure_ntff_hook():
    """The agent image's antenv lacks axon_hooks; synthesize it from the boot
    helper so trace=True can profile NTFF via the axon .so. Best-effort."""
    import sys
    import types

    try:
        import antenv.axon_hooks  # noqa: F401
        return
    except ImportError:
        pass
    try:
        import antenv
        from trn_agent_boot.trn_boot import _ntff_profile_via_ctypes

        mod = types.ModuleType("antenv.axon_hooks")
        mod._hook = None

        def set_axon_ntff_profile_hook(h):
            mod._hook = h

        def get_axon_ntff_profile_hook():
            return mod._hook

        mod.set_axon_ntff_profile_hook = set_axon_ntff_profile_hook
        mod.get_axon_ntff_profile_hook = get_axon_ntff_profile_hook
        sys.modules["antenv.axon_hooks"] = mod
        antenv.axon_hooks = mod
        import os

        for so_path in (
            "/opt/axon/libaxon_pjrt.so",
            "/root/.axon_site/libaxon_pjrt.so",
        ):
            if os.path.exists(so_path):
                hook = _ntff_profile_via_ctypes(so_path)
                if hook is not None:
                    mod._hook = hook
                break
    except Exception:
        pass


def _run(inputs, trace=False, trace_kwargs=None):
    from concourse.bass_utils import run_bass_kernel_spmd

    _ensure_ntff_hook()

    if "nc" not in _CACHE:
        _CACHE["nc"] = _build_program()
    nc = _CACHE["nc"]

    x = np.asarray(inputs["x"], dtype=np.float32)
    assert x.shape == (B, S, D), x.shape
    wqk, wv, bqk, bvv, tri, ident = _prep_shared(
        np.asarray(inputs["Wq"]), np.asarray(inputs["bq"]),
        np.asarray(inputs["Wk"]), np.asarray(inputs["bk"]),
        np.asarray(inputs["Wv"]), np.asarray(inputs["bv"]),
    )

    # x [B, S, D] -> bf16 -> [B, NCH, 128, S]  (xt[b, c, p, s] = x[b, s, 128c+p])
    x_bf = x.astype(BF16)
    xt_all = np.ascontiguousarray(
        x_bf.reshape(B, S, NCH, 128).transpose(0, 2, 3, 1)
    )

    in_maps = []
    for core in range(NCORES):
        in_maps.append(
            {
                "xt": xt_all[core * BPC : (core + 1) * BPC],
                "wqk": wqk,
                "wv": wv,
                "bqk": bqk,
                "bv": bvv,
                "tri": tri,
                "ident": ident,
            }
        )

    kwargs = {}
    if trace:
        kwargs["trace"] = True
        if trace_kwargs:
            kwargs["trace_kwargs"] = trace_kwargs
    bkr = run_bass_kernel_spmd(nc, in_maps, core_ids=list(range(NCORES)), **kwargs)

    outs = np.stack([np.asarray(r["out"]) for r in bkr.results])  # [8, BPC, F+1, S]
    out_aug = outs.reshape(B, F + 1, S).astype(np.float32)
    out = out_aug[:, :F, :] / out_aug[:, F : F + 1, :]
    return np.ascontiguousarray(out.transpose(0, 2, 1)), bkr


def kernel(**inputs) -> np.ndarray:
    out, _ = _run(inputs, trace=False)
    return out


# revision 7
# speedup vs baseline: 1.1707x; 1.1707x over previous
"""Trainium2 Bass kernel: single-head causal attention, data-parallel over batch.

Reference computation (per batch b):
    q = x @ Wq + bq; k = x @ Wk + bk; v = x @ Wv + bv       # [S, F]
    s = (q @ k^T) / sqrt(D)   (causal masked)               # [S, S]
    p = softmax(s, axis=-1)
    out = p @ v                                             # [S, F]

Shapes: B=32, S=1024, D=1024, F=64.  8 cores, 4 batches/core.

Device-side layout strategy (per core, per batch):
  - host provides x pre-transposed: XT[d, s] (bf16), chunked [8, 128, 1024]
  - pass1: psum_qk[128, S] = [Wq | Wk]^T-chunks (stationary, M=128) vs XT (moving)
           -> partitions 0-63 = Q^T, 64-127 = K^T  (full-rate M=128)
  - pass2: psum_v[64, S]  = Wv-chunks vs XT  -> V^T
  - V^T -> V via 8 PE transposes; V augmented with a ones column (M=65)
  - scores computed TRANSPOSED: ST[k, q] = K^T-chunk (stationary) vs Q^T (moving)
    so softmax probs P^T = exp(ST) feed the P@V matmul directly as moving operand
  - causal: only q >= 128*t computed per k-chunk t; diagonal 128x128 tile
    masked by multiplying exp() output with an upper-triangular 0/1 mask (GPSIMD)
  - P@V: out_aug[65, q] accumulates over k-chunks; row 64 = sum(exp) (softmax denom)
  - host divides by the denominator row and transposes back.
"""

import numpy as np
import ml_dtypes

BF16 = ml_dtypes.bfloat16

B, S, D, F = 32, 1024, 1024, 64
NCORES = 8
BPC = B // NCORES          # batches per core
NCH = D // 128             # contraction chunks (8)

_CACHE = {}


def _build_program():
    from contextlib import ExitStack

    import concourse.bacc as bacc
    import concourse.tile as tile
    from concourse import mybir

    f32 = mybir.dt.float32
    bf16 = mybir.dt.bfloat16
    Exp = mybir.ActivationFunctionType.Exp

    nc = bacc.Bacc(None, target_bir_lowering=False, debug=False)

    xt_d = nc.declare_dram_parameter("xt", [BPC, NCH, 128, S], bf16, isOutput=False)
    wqk_d = nc.declare_dram_parameter("wqk", [NCH, 128, 128], bf16, isOutput=False)
    wv_d = nc.declare_dram_parameter("wv", [NCH, 128, F], bf16, isOutput=False)
    bqk_d = nc.declare_dram_parameter("bqk", [128, 1], f32, isOutput=False)
    bv_d = nc.declare_dram_parameter("bv", [F, 1], f32, isOutput=False)
    tri_d = nc.declare_dram_parameter("tri", [128, 128], bf16, isOutput=False)
    ident_d = nc.declare_dram_parameter("ident", [F, F], bf16, isOutput=False)
    out_d = nc.declare_dram_parameter("out", [BPC, F + 1, S], f32, isOutput=True)

    with tile.TileContext(nc) as tc, ExitStack() as ctx:
        singles = ctx.enter_context(tc.tile_pool(name="singles", bufs=1))
        xt_pool = ctx.enter_context(tc.tile_pool(name="xt", bufs=2))
        qkv_pool = ctx.enter_context(tc.tile_pool(name="qkv", bufs=2))
        pt_pool = ctx.enter_context(tc.tile_pool(name="pt", bufs=4))
        osb_pool = ctx.enter_context(tc.tile_pool(name="osb", bufs=2))
        # PSUM budget (8 banks of 2KB/partition):
        #   proj pool: 2 slots x 2 banks (qk [128,1024]f32 / v / vt rotate)
        #   s pool:    2 slots x 1 bank  (per-segment score tiles [128,512]f32)
        #   o pool:    1 slot  x 2 banks ([65,1024]f32 accumulator)
        pp_pool = ctx.enter_context(tc.tile_pool(name="pp", bufs=2, space="PSUM"))
        ps_pool = ctx.enter_context(tc.tile_pool(name="ps", bufs=2, space="PSUM"))
        po_pool = ctx.enter_context(tc.tile_pool(name="po", bufs=1, space="PSUM"))

        # --- load constants / weights once ---
        wqk_sb = singles.tile([128, NCH, 128], bf16, tag="wqk")
        nc.sync.dma_start(out=wqk_sb, in_=wqk_d[:, :, :].rearrange("c p m -> p c m"))
        wv_sb = singles.tile([128, NCH, F], bf16, tag="wv")
        nc.sync.dma_start(out=wv_sb, in_=wv_d[:, :, :].rearrange("c p m -> p c m"))
        bqk_sb = singles.tile([128, 1], f32, tag="bqk")
        nc.sync.dma_start(out=bqk_sb, in_=bqk_d[:, :])
        bv_sb = singles.tile([F, 1], f32, tag="bv")
        nc.sync.dma_start(out=bv_sb, in_=bv_d[:, :])
        tri_sb = singles.tile([128, 128], bf16, tag="tri")
        nc.sync.dma_start(out=tri_sb, in_=tri_d[:, :])
        ident_sb = singles.tile([F, F], bf16, tag="ident")
        nc.sync.dma_start(out=ident_sb, in_=ident_d[:, :])

        for b in range(BPC):
            # --- load XT for this batch (two halves so matmuls start earlier) ---
            xt = xt_pool.tile([128, NCH, S], bf16, tag="xt")
            nc.sync.dma_start(
                out=xt[:, :, 0:512],
                in_=xt_d[b, :, :, 0:512].rearrange("c p s -> p c s"),
            )
            nc.sync.dma_start(
                out=xt[:, :, 512:1024],
                in_=xt_d[b, :, :, 512:1024].rearrange("c p s -> p c s"),
            )

            # --- pass1: Q^T (parts 0-63) and K^T (parts 64-127) ---
            psum_qk = pp_pool.tile([128, S], f32, tag="pp")
            for sj in range(2):
                for c in range(NCH):
                    nc.tensor.matmul(
                        out=psum_qk[:, 512 * sj : 512 * sj + 512],
                        lhsT=wqk_sb[:, c, :],
                        rhs=xt[:, c, 512 * sj : 512 * sj + 512],
                        start=(c == 0),
                        stop=(c == NCH - 1),
                    )
            qt = qkv_pool.tile([64, S], bf16, tag="qt")
            nc.vector.tensor_scalar_add(out=qt, in0=psum_qk[0:64, :], scalar1=bqk_sb[0:64, :])
            kt = qkv_pool.tile([64, S], bf16, tag="kt")
            nc.vector.tensor_scalar_add(out=kt, in0=psum_qk[64:128, :], scalar1=bqk_sb[64:128, :])

            # --- pass2: V^T ---
            psum_v = pp_pool.tile([64, S], f32, tag="pp")
            for sj in range(2):
                for c in range(NCH):
                    nc.tensor.matmul(
                        out=psum_v[:, 512 * sj : 512 * sj + 512],
                        lhsT=wv_sb[:, c, :],
                        rhs=xt[:, c, 512 * sj : 512 * sj + 512],
                        start=(c == 0),
                        stop=(c == NCH - 1),
                    )
            vt = qkv_pool.tile([64, S], bf16, tag="vt")
            nc.vector.tensor_scalar_add(out=vt, in0=psum_v[:, :], scalar1=bv_sb)

            # --- V^T -> V (PE transposes), augmented with ones column ---
            psum_vt = pp_pool.tile([128, NCH, F], bf16, tag="pp")
            for t in range(NCH):
                nc.tensor.transpose(
                    out=psum_vt[:, t, :],
                    in_=vt[:, 128 * t : 128 * t + 128],
                    identity=ident_sb,
                )
            v_aug = qkv_pool.tile([128, NCH, F + 1], bf16, tag="vaug")
            nc.gpsimd.memset(v_aug[:, :, F : F + 1], 1.0)
            nc.vector.tensor_copy(out=v_aug[:, :, 0:F], in_=psum_vt[:, :, :])

            # --- attention: k-chunk t covers queries q >= 128*t ---
            psum_o = po_pool.tile([F + 1, S], f32, tag="po")
            for t in range(NCH):
                q0 = 128 * t
                segs = []
                for j in range(2):
                    g0 = max(512 * j, q0)
                    g1 = 512 * (j + 1)
                    if g0 < g1:
                        segs.append((j, g0, g1))

                for si, (j, g0, g1) in enumerate(segs):
                    L = g1 - g0
                    psum_s = ps_pool.tile([128, 512], f32, tag="ps")
                    nc.tensor.matmul(
                        out=psum_s[:, 0:L],
                        lhsT=kt[:, q0 : q0 + 128],
                        rhs=qt[:, g0:g1],
                        start=True,
                        stop=True,
                    )
                    pt = pt_pool.tile([128, 512], bf16, tag="pt")
                    nc.scalar.activation(out=pt[:, 0:L], in_=psum_s[:, 0:L], func=Exp)
                    if si == 0:
                        # causal mask on the diagonal 128x128 tile (0/1 upper-tri)
                        nc.gpsimd.tensor_mul(
                            out=pt[:, 0:128], in0=pt[:, 0:128], in1=tri_sb
                        )
                    nc.tensor.matmul(
                        out=psum_o[:, g0:g1],
                        lhsT=v_aug[:, t, :],
                        rhs=pt[:, 0:L],
                        start=(t == 0),
                        stop=(t == (3 if j == 0 else 7)),
                    )

            out_sb = osb_pool.tile([F + 1, S], f32, tag="osb")
            nc.vector.tensor_copy(out=out_sb, in_=psum_o)
            nc.sync.dma_start(out=out_d[b], in_=out_sb)

    if not nc.is_finalized():
        nc.finalize()
    return nc


def _prep_shared(Wq, bq, Wk, bk, Wv, bv):
    scale = 1.0 / np.sqrt(np.float32(D))
    wqks = np.concatenate([Wq.astype(np.float32) * scale, Wk.astype(np.float32)], axis=1)  # [D, 128]
    wqk = np.ascontiguousarray(
        wqks.reshape(NCH, 128, 128)
    ).astype(BF16)  # [c, p, m]
    wv = np.ascontiguousarray(
        Wv.astype(np.float32).reshape(NCH, 128, F)
    ).astype(BF16)
    bqk = np.concatenate(
        [bq.astype(np.float32) * scale, bk.astype(np.float32)]
    ).reshape(128, 1)
    bvv = bv.astype(np.float32).reshape(F, 1)
    tri = np.triu(np.ones((128, 128), np.float32)).astype(BF16)
    ident = np.eye(F, dtype=np.float32).astype(BF16)
    return wqk, wv, bqk, bvv, tri, ident


def _ensure_ntff_hook():
    """The agent image's antenv lacks axon_hooks; synthesize it from the boot
    helper so trace=True can profile NTFF via the axon .so. Best-effort."""
    import sys
    import types

    try:
        import antenv.axon_hooks  # noqa: F401
        return
    except ImportError:
        pass
    try:
        import antenv
        from trn_agent_boot.trn_boot import _ntff_profile_via_ctypes

        mod = types.ModuleType("antenv.axon_hooks")
        mod._hook = None

        def set_axon_ntff_profile_hook(h):
            mod._hook = h

        def get_axon_ntff_profile_hook():
            return mod._hook

        mod.set_axon_ntff_profile_hook = set_axon_ntff_profile_hook
        mod.get_axon_ntff_profile_hook = get_axon_ntff_profile_hook
        sys.modules["antenv.axon_hooks"] = mod
        antenv.axon_hooks = mod
        import os

        for so_path in (
            "/opt/axon/libaxon_pjrt.so",
            "/root/.axon_site/libaxon_pjrt.so",
        ):
            if os.path.exists(so_path):
                hook = _ntff_profile_via_ctypes(so_path)
                if hook is not None:
                    mod._hook = hook
                break
    except Exception:
        pass


def _run(inputs, trace=False, trace_kwargs=None):
    from concourse.bass_utils import run_bass_kernel_spmd

    _ensure_ntff_hook()

    if "nc" not in _CACHE:
        _CACHE["nc"] = _build_program()
    nc = _CACHE["nc"]

    x = np.asarray(inputs["x"], dtype=np.float32)
    assert x.shape == (B, S, D), x.shape
    wqk, wv, bqk, bvv, tri, ident = _prep_shared(
        np.asarray(inputs["Wq"]), np.asarray(inputs["bq"]),
        np.asarray(inputs["Wk"]), np.asarray(inputs["bk"]),
        np.asarray(inputs["Wv"]), np.asarray(inputs["bv"]),
    )

    # x [B, S, D] -> bf16 -> [B, NCH, 128, S]  (xt[b, c, p, s] = x[b, s, 128c+p])
    x_bf = x.astype(BF16)
    xt_all = np.ascontiguousarray(
        x_bf.reshape(B, S, NCH, 128).transpose(0, 2, 3, 1)
    )

    in_maps = []
    for core in range(NCORES):
        in_maps.append(
            {
                "xt": xt_all[core * BPC : (core + 1) * BPC],
                "wqk": wqk,
                "wv": wv,
                "bqk": bqk,
                "bv": bvv,
                "tri": tri,
                "ident": ident,
            }
        )

    kwargs = {}
    if trace:
        kwargs["trace"] = True
        if trace_kwargs:
            kwargs["trace_kwargs"] = trace_kwargs
    bkr = run_bass_kernel_spmd(nc, in_maps, core_ids=list(range(NCORES)), **kwargs)

    outs = np.stack([np.asarray(r["out"]) for r in bkr.results])  # [8, BPC, F+1, S]
    out_aug = outs.reshape(B, F + 1, S).astype(np.float32)
    out = out_aug[:, :F, :] / out_aug[:, F : F + 1, :]
    return np.ascontiguousarray(out.transpose(0, 2, 1)), bkr


def kernel(**inputs) -> np.ndarray:
    out, _ = _run(inputs, trace=False)
    return out
